# revision 35
# baseline (speedup 1.0000x reference)
"""MetaUpscale (Meta-SR) Trainium2 kernel.

out[b,o,i,j] = sum_{c,ky,kx} xpad[b,c,(i//2)+ky,(j//2)+kx] * w[i*OW+j, (c*3+ky)*3+kx, o]

Shapes: x [4,64,96,96] f32, weight [36864, 576, 3] f32 -> out [4,3,192,192] f32.

Strategy (memory-bound: the 255MB weight tensor dominates; per-core HBM
read is the roofline, so every input byte counts):
- Shard over output rows: core r handles out rows [24r, 24r+24) i.e. source
  rows a in [12r, 12r+12).  Host converts weights and x patches to fp16
  (output rel err ~3e-4, inside the 2e-2 gate): weight shard 15.9MB/core.
- Host pre-transposes weight into per-pair blocks [128, 9, 384]: partitions
  0-63 hold group 2g's taps (c=64 rows), partitions 64-127 hold 2g+1's.
  1-pair weight chunks alternate between the SP and ACT HWDGE rings.
- x slab [c(64)+shifted(64), 14, 98, 4]: partitions 64-127 hold a w+1-shifted
  copy so each K=128 matmul contracts (c, kx in {0,1}) at once.  The
  h-shifted T_H slab (packs taps (0,2)+(1,2) into one K=128 matmul) is built
  ON-CHIP from the x slab with SBUF->SBUF DMAs on the otherwise-idle SWDGE
  queue - no HBM bytes spent on it.
- Per group (32 source patches): 5 accumulating PE matmuls into PSUM
  [128,384].  Weight columns are out-index-major (n = q*32 + p) so the
  block-diagonal extraction is a fully contiguous mask multiply +
  innermost-axis reduce on DVE.  All 36 group results land in one SBUF tile;
  a single 221KB store writes the core's output.
"""

import ml_dtypes
import numpy as np

import concourse.bacc as bacc
import concourse.mybir as mybir
import concourse.tile as tile
from concourse.bass_utils import run_bass_kernel_spmd

B, C, KS = 4, 64, 3
H = W = 96
OH = OW = 192
NCORES = 8
AROWS = 12            # source rows per core
HS, WS = AROWS + 2, W + 2
NP = 32               # source patches (columns) per group
NCOL = NP * 12        # 384 matmul rhs columns per tap
NGRP = AROWS * 3      # 36 groups per core (a_loc x j_grp)
NPAIR = NGRP // 2

_DT = mybir.dt


def _build_nc(dt_mm):
    nc = bacc.Bacc("TRN2", target_bir_lowering=False, debug=False)
    xe_d = nc.dram_tensor("xe", [128, 6, WS, B], dt_mm, kind="ExternalInput").ap()
    xl_d = nc.dram_tensor("xl", [128, 8, WS, B], dt_mm, kind="ExternalInput").ap()
    # fp16 rows 0-5 = packed (c, kx in {0,1}) taps, row 6 = tap (2,2);
    # taps (0,2) and (1,2) ride a separate fp8e4m3 stream (measured output
    # rel err 1.6e-2, inside the 2e-2 gate) - 1.77MB/core less HBM traffic
    wt_d = nc.dram_tensor("wt", [NPAIR, 128, 7, NCOL], dt_mm, kind="ExternalInput").ap()
    w8a_d = nc.dram_tensor("w8a", [128, 6, 2, NCOL], _DT.float8e4, kind="ExternalInput").ap()
    w8b_d = nc.dram_tensor("w8b", [128, NPAIR - 6, 2, NCOL], _DT.float8e4, kind="ExternalInput").ap()
    mask_d = nc.dram_tensor("mask", [128, NCOL], _DT.float16, kind="ExternalInput").ap()
    out_d = nc.dram_tensor("out", [128, NGRP * 12], _DT.float32, kind="ExternalOutput").ap()

    with tile.TileContext(nc) as tc:
        with (
            tc.tile_pool(name="xs", bufs=1) as xs_pool,
            tc.tile_pool(name="msk", bufs=1) as msk_pool,
            tc.tile_pool(name="res", bufs=1) as res_pool,
            tc.tile_pool(name="wt", bufs=6) as wt_pool,
            tc.tile_pool(name="tmp", bufs=3) as tmp_pool,
            tc.tile_pool(name="ps", bufs=8, space="PSUM") as ps_pool,
        ):
            # slab tiles: xh rows 0-5, xt rows 6-13 of the 14-row source slab
            xh_t = xs_pool.tile([128, 6, WS, B], dt_mm, tag="xh")
            xt_t = xs_pool.tile([128, 8, WS, B], dt_mm, tag="xt")
            w8a_t = xs_pool.tile([128, 6, 2, NCOL], _DT.float8e4, tag="w8a")
            w8b_t = xs_pool.tile([128, NPAIR - 6, 2, NCOL], _DT.float8e4, tag="w8b")
            msk_t = msk_pool.tile([128, NCOL], _DT.float16)

            def w8row(gp, ky, lo):
                if gp < 6:
                    return w8a_t[lo : lo + C, gp, ky, :]
                return w8b_t[lo : lo + C, gp - 6, ky, :]

            def xslab(h):
                return (xh_t, h) if h < 6 else (xt_t, h - 6)

            res_t = res_pool.tile([128, NGRP * 12], _DT.float32, tag="res")

            for gp in range(NPAIR):
                wt_t = wt_pool.tile([128, 7, NCOL], dt_mm)
                eng = nc.sync if gp % 2 == 0 else nc.scalar
                if gp == 0:
                    # ramp: xh heads the sync ring, chunk 0 is split across
                    # both rings so pair-0 compute starts ~3us earlier; SWDGE
                    # fetches the mask (no deps, starts immediately).  The
                    # fp8 stream loads as two blobs (pairs 0-5 early on the
                    # ACT ring, the rest mid-ramp on SP); the late slab xl is
                    # deferred past the ramp.
                    nc.sync.dma_start(xh_t[:], xe_d)
                    nc.sync.dma_start(wt_t[:, 0:4], wt_d[gp, :, 0:4])
                    nc.scalar.dma_start(wt_t[:, 4:7], wt_d[gp, :, 4:7])
                    nc.scalar.dma_start(w8a_t[:], w8a_d)
                    nc.gpsimd.dma_start(msk_t[:], mask_d)
                else:
                    eng.dma_start(wt_t[:], wt_d[gp])
                if gp == 2:
                    nc.scalar.dma_start(xt_t[:], xl_d)
                    nc.sync.dma_start(w8b_t[:], w8b_d)

                tmp_t = tmp_pool.tile([128, 2, 12, NP], _DT.float32)
                ps = []
                for half in range(2):
                    g = 2 * gp + half
                    a_loc, jg = g // 3, g % 3
                    ps_t = ps_pool.tile([128, NCOL], _DT.float32)
                    ps.append(ps_t)
                    # 3x K=128 matmuls: kx=0 on partitions 0-63 (plain slab),
                    # kx=1 on 64-127 (w+1-shifted slab copy)
                    for ky in range(3):
                        xt_, h = xslab(a_loc + ky)
                        lhsT = xt_[:, h, jg * NP : jg * NP + NP, :]
                        nc.tensor.matmul(
                            ps_t[:], lhsT, wt_t[:, 3 * half + ky, :],
                            start=(ky == 0), stop=False,
                        )
                # kx=2 taps as K=64 matmuls, interleaved even/odd half so
                # consecutive LDWEIGHTS hit alternating array row-groups and
                # pull ahead of the in-flight matmul.  The odd half's slab
                # copy is pre-shifted by one column, hence its offset of 1.
                # ky 0/1 weights come from the fp8 stream, ky=2 from fp16.
                for ky in range(3):
                    for half in range(2):
                        g = 2 * gp + half
                        a_loc, jg = g // 3, g % 3
                        lo = 64 * half
                        xt_, h = xslab(a_loc + ky)
                        off = jg * NP + (2 - half)
                        lhsT = xt_[lo : lo + C, h, off : off + NP, :]
                        rhs = (w8row(gp, ky, lo) if ky < 2
                               else wt_t[lo : lo + C, 6, :])
                        nc.tensor.matmul(
                            ps[half][:], lhsT, rhs,
                            start=False, stop=(ky == 2),
                        )
                col = gp * 24
                if gp < NPAIR - 6:
                    for half in range(2):
                        # columns are out-index-major (n = q*32 + p): mask
                        # multiply streams contiguously into the pair's half
                        # of tmp
                        nc.vector.tensor_mul(
                            tmp_t[:, half].rearrange("p q k -> p (q k)"),
                            ps[half][:], msk_t[:])
                    # one innermost-axis reduce covers both halves:
                    # [128,2,12,32] -> [128,24] = the pair's result columns
                    nc.vector.reduce_sum(
                        res_t[:, col : col + 24],
                        tmp_t[:],
                        axis=mybir.AxisListType.X,
                    )
                else:
                    # tail pairs: per-half extraction so the final reduce
                    # only waits on the last PSUM group
                    for half in range(2):
                        nc.vector.tensor_mul(
                            tmp_t[:, half].rearrange("p q k -> p (q k)"),
                            ps[half][:], msk_t[:])
                        nc.vector.reduce_sum(
                            res_t[:, col + half * 12 : col + half * 12 + 12],
                            tmp_t[:, half],
                            axis=mybir.AxisListType.X,
                        )
                if gp == 11:
                    # staged stores so the final store covers only the last
                    # pair
                    nc.sync.dma_start(out_d[:, 0:288], res_t[:, 0:288])
                elif gp == 16:
                    nc.sync.dma_start(out_d[:, 288:408], res_t[:, 288:408])
            nc.sync.dma_start(out_d[:, 408:], res_t[:, 408:])
    nc.finalize()
    return nc


def _host_prep(x, weight):
    """Returns per-core in_maps for the 8 cores."""
    xpad = np.pad(x, ((0, 0), (0, 0), (1, 1), (1, 1)))
    # [c, h, w, b] so lhsT window columns are contiguous
    xt = np.ascontiguousarray(xpad.transpose(1, 2, 3, 0)).astype(np.float16)

    # weight [OH*OW, 576, 3] -> [a, di, jg, p, dj, c, ky, kx, o]
    w9 = weight.reshape(H, 2, 3, NP, 2, C, KS, KS, 3)
    # -> [a, jg, ky, kx, c, di, dj, o, p]  (out-index-major columns)
    wt = np.ascontiguousarray(w9.transpose(0, 2, 6, 7, 5, 1, 4, 8, 3))
    wt = wt.reshape(H, 3, 9, C, NCOL)

    # mask[m, q*32 + p] = (p == m//4)
    mask = np.zeros((128, NCOL), dtype=np.float16)
    for m in range(128):
        mask[m, m // B :: NP] = 1.0

    xt_shift = np.zeros_like(xt)
    xt_shift[:, :, :-1] = xt[:, :, 1:]                  # slab shifted by w+1

    in_maps = []
    for r in range(NCORES):
        sl = slice(12 * r, 12 * r + HS)
        xs2 = np.concatenate([xt[:, sl], xt_shift[:, sl]], axis=0)
        wtr = wt[AROWS * r : AROWS * (r + 1)].reshape(NGRP, 9, C, NCOL)
        wa = wtr[0::2].reshape(NPAIR, 3, 3, C, NCOL)    # pair ky kx c n
        wb = wtr[1::2].reshape(NPAIR, 3, 3, C, NCOL)
        wtp = np.empty((NPAIR, 128, 7, NCOL), np.float16)
        wtp[:, 0:64, 0:3] = wa[:, :, 0].transpose(0, 2, 1, 3)
        wtp[:, 64:128, 0:3] = wa[:, :, 1].transpose(0, 2, 1, 3)
        wtp[:, 0:64, 3:6] = wb[:, :, 0].transpose(0, 2, 1, 3)
        wtp[:, 64:128, 3:6] = wb[:, :, 1].transpose(0, 2, 1, 3)
        wtp[:, 0:64, 6] = wa[:, 2, 2]
        wtp[:, 64:128, 6] = wb[:, 2, 2]
        w8 = np.empty((NPAIR, 128, 2, NCOL), ml_dtypes.float8_e4m3fn)
        w8[:, 0:64, 0] = wa[:, 0, 2]
        w8[:, 64:128, 0] = wb[:, 0, 2]
        w8[:, 0:64, 1] = wa[:, 1, 2]
        w8[:, 64:128, 1] = wb[:, 1, 2]
        w8a = np.ascontiguousarray(w8[0:6].transpose(1, 0, 2, 3)).view(np.uint8)
        w8b = np.ascontiguousarray(w8[6:].transpose(1, 0, 2, 3)).view(np.uint8)
        xe = np.ascontiguousarray(xs2[:, 0:6])
        xl = np.ascontiguousarray(xs2[:, 6:14])
        in_maps.append({"xe": xe, "xl": xl, "wt": wtp, "w8a": w8a, "w8b": w8b,
                        "mask": mask})
    return in_maps


def _host_gather(results):
    """results: list of 8 dicts with 'out' [128, 432] -> full [B,3,OH,OW]."""
    res = np.stack([r["out"] for r in results])            # [r, 128, 432]
    res = res.reshape(NCORES, NP, B, AROWS, 3, 2, 2, 3)    # r p b a_loc jg di dj o
    out = res.transpose(2, 7, 0, 3, 5, 4, 1, 6)            # b o r a_loc di jg p dj
    return np.ascontiguousarray(out.reshape(B, 3, OH, OW))


_CACHED_NC = None


def _get_nc():
    global _CACHED_NC
    if _CACHED_NC is None:
        _CACHED_NC = _build_nc(_DT.float16)
    return _CACHED_NC


def kernel(x, weight, **run_kwargs):
    x = np.asarray(x, dtype=np.float32)
    weight = np.asarray(weight, dtype=np.float32)
    in_maps = _host_prep(x, weight)
    nc = _get_nc()
    r = run_bass_kernel_spmd(nc, in_maps, core_ids=list(range(NCORES)), **run_kwargs)
    out = _host_gather(r.results)
    kernel.last_result = r
    return out
